# revision 3
# baseline (speedup 1.0000x reference)
"""Trainium2 Bass kernel for nn_C3AH (C3-style hypergraph attention block).

Contract: kernel(**inputs) takes the FULL unsharded inputs (numpy f32) and
returns the FULL output [16, 256, 64, 64] f32.  Internally: data-parallel over
batch across 8 NeuronCores (2 batches per core), weights replicated, bf16
matmuls with f32 PSUM accumulation.

v3 — ACT-roofline rewrite.  Design notes (per core, 2 batches):
  - ACT (ScalarE) is the hard floor: 4 full-tensor activations
    (cv1/cv2 SiLU, node GELU, cv3 SiLU) = 8.4M elems @ 1.2GHz ~= 55us,
    + exp + 4 table loads.  Everything else is scheduled to hide under it.
  - Convs: [128, 2048] PSUM tiles (4 banks), kt-outer/ns-inner so the
    stationary operand is reused across 4 consecutive matmuls; one ACT call
    per 2048 chunk (amortizes the 352-cycle ACTIVATE overhead).
  - ctx offsets: pre_w is folded into ctx_w on the host (G = pre_w^T-app),
    so q^T = q0T + G-matmul(ctx).  The G matmul runs with ctx as the
    2-column stationary (LDWEIGHTS ~= 2 cols) and offsets come out natural
    [2, 2048]; a single xbar DMA transpose puts them back in [c-part, (e,m)]
    layout.  This kills the 64 LD-bound 128-col stationary loads the
    transposed-orientation matmul would need.
  - He aggregation flipped: stationary = P-block [128n, 8e] (8-col
    LDWEIGHTS ~= 7ns), moving = tokens^T blocks (both m-halves per matmul
    via a strided 3D AP).  He comes out natural [8, 256]; a PE-mode
    transpose (vs identity) flips it for the edge linear.
  - softmax: no-max-sub exp over [40, 4096] in ONE call with accum_out=Z;
    1/Z is folded into the He psum->sbuf copy and the whTT copy
    (per-partition tensor_scalar), so nothing normalizes the big A tensor.
  - node-apply packs b0/b1 in PE row-groups 0/32 (tile_position).
  - Engine balance: plain DMA on gpsimd (SWDGE), transposes on sync
    (HWDGE), PSUM drains on DVE.  ACT table sequence SILU->EXP->GELU->SILU
    (4 loads, minimal for this op sandwich).
"""
import sys
import functools

sys.path.insert(0, "/opt/trn_rl_repo")

import numpy as np
import ml_dtypes

import concourse.bass as bass
import concourse.tile as tile
from concourse import bacc, mybir
from concourse.bass_utils import run_bass_kernel_spmd

BF16 = ml_dtypes.bfloat16
FP32 = mybir.dt.float32
BF = mybir.dt.bfloat16
AF = mybir.ActivationFunctionType
AX = mybir.AxisListType

B, C1, H, W = 16, 256, 64, 64
N = H * W            # 4096
CH, C2, E = 256, 256, 8
NCORES = 8
BLOC = B // NCORES   # 2 batches per core
EPS = 1e-5
LSCALE = 1.0 / 64.0  # 1/(NH*sqrt(HD))

NCH = 2048           # free-dim chunk for conv PSUM tiles / ACT calls
NSUB = 512           # matmul moving-operand max (one PSUM bank fp32)
NCHUNKS = N // NCH   # 2


def emit_kernel(nc):
    # ---------------- DRAM I/O ----------------
    x_d = nc.dram_tensor("x", [BLOC, C1, N], BF, kind="ExternalInput")
    w1t_d = nc.dram_tensor("w1t", [C1, CH], BF, kind="ExternalInput")
    w2t_d = nc.dram_tensor("w2t", [C1, CH], BF, kind="ExternalInput")
    w3t_d = nc.dram_tensor("w3t", [2 * CH, C2], BF, kind="ExternalInput")
    gwt_d = nc.dram_tensor("gwt", [2 * CH, E * CH], BF, kind="ExternalInput")
    q0t_d = nc.dram_tensor("q0t", [128, 16], BF, kind="ExternalInput")
    ident_d = nc.dram_tensor("ident", [8, 8], BF, kind="ExternalInput")
    edgewt_d = nc.dram_tensor("edgewt", [CH, CH], BF, kind="ExternalInput")
    nodewt_d = nc.dram_tensor("nodewt", [CH, CH], BF, kind="ExternalInput")
    b1_d = nc.dram_tensor("b1", [CH], FP32, kind="ExternalInput")
    b2_d = nc.dram_tensor("b2", [CH], FP32, kind="ExternalInput")
    b3_d = nc.dram_tensor("b3", [C2], FP32, kind="ExternalInput")
    eb_d = nc.dram_tensor("eb", [CH], FP32, kind="ExternalInput")
    nb_d = nc.dram_tensor("nb", [CH], FP32, kind="ExternalInput")
    out_d = nc.dram_tensor("out", [BLOC, C2, N], BF, kind="ExternalOutput")

    with tile.TileContext(nc) as tc:
        emit_body(nc, tc, dict(
            x=x_d, w1t=w1t_d, w2t=w2t_d, w3t=w3t_d, gwt=gwt_d, q0t=q0t_d,
            ident=ident_d, edgewt=edgewt_d, nodewt=nodewt_d,
            b1=b1_d, b2=b2_d, b3=b3_d, eb=eb_d, nb=nb_d, out=out_d))
    return nc


def emit_body(nc, tc, d):
    from contextlib import ExitStack
    ctx = ExitStack()
    with ctx:
        singles = ctx.enter_context(tc.tile_pool(name="singles", bufs=1))
        xs_pool = ctx.enter_context(tc.tile_pool(name="xs", bufs=2))
        tok_pool = ctx.enter_context(tc.tile_pool(name="tok", bufs=2))
        y2_pool = ctx.enter_context(tc.tile_pool(name="y2", bufs=2))
        l2_pool = ctx.enter_context(tc.tile_pool(name="l2", bufs=2))
        sm_pool = ctx.enter_context(tc.tile_pool(name="sm", bufs=1))
        small = ctx.enter_context(tc.tile_pool(name="small", bufs=2))
        stage = ctx.enter_context(tc.tile_pool(name="stage", bufs=3))
        # PSUM: 2 x [128, 2048] f32 = 2 x 4 banks = all 8 banks.  Small
        # attention-phase tiles rotate through the same ring.
        psum = ctx.enter_context(tc.tile_pool(name="psum", bufs=2, space="PSUM"))

        # ---------------- loads ----------------
        def ld_w(name, dram, kt, mcols, eng):
            t = singles.tile([128, kt, mcols], BF, tag=name)
            eng.dma_start(out=t, in_=dram[:].rearrange("(t p) m -> p t m", p=128))
            return t

        def ld_b(name, dram, eng):
            t = singles.tile([128, 2], FP32, tag=name)
            eng.dma_start(out=t, in_=dram[:].rearrange("(t p) -> p t", p=128))
            return t

        w1t = ld_w("w1t", d["w1t"], 2, CH, nc.gpsimd)
        b1s = ld_b("b1", d["b1"], nc.gpsimd)

        xs = [xs_pool.tile([128, 2, N], BF, tag="xs", name="xs") for _ in range(BLOC)]
        for b in range(BLOC):
            xr = d["x"][b].rearrange("(t p) n -> p t n", p=128)
            for c2 in range(NCHUNKS):
                nc.sync.dma_start(out=xs[b][:, :, c2 * NCH:(c2 + 1) * NCH],
                                  in_=xr[:, :, c2 * NCH:(c2 + 1) * NCH])

        w2t = ld_w("w2t", d["w2t"], 2, CH, nc.gpsimd)
        b2s = ld_b("b2", d["b2"], nc.gpsimd)

        # persistent activation-side tiles
        tokens = [tok_pool.tile([128, 2, N], BF, tag="tok", name="tok") for _ in range(BLOC)]
        y2 = [y2_pool.tile([128, 2, N], BF, tag="y2", name="y2") for _ in range(BLOC)]
        # tokens^T per batch: tl2[b][:, m*32+t, :] = tokens[b][:, m, 128t:128t+128]^T
        tl2 = [l2_pool.tile([128, 64, 128], BF, tag="l2", name="l2") for _ in range(BLOC)]
        tok_sums = [small.tile([128, 2, NCHUNKS], FP32, tag="tsum", name="tsum") for _ in range(BLOC)]
        maxp = [small.tile([128, 2, NCHUNKS], FP32, tag="maxp", name="maxp") for _ in range(BLOC)]

        # softmax / attention state (batch b at partition rows [32b, 32b+8))
        lgs = sm_pool.tile([40, N], BF, tag="lgs", name="lgs")
        Pn = sm_pool.tile([48, N], BF, tag="Pn", name="Pn")
        PT = sm_pool.tile([128, 32, 32], BF, tag="PT", name="PT")
        offn = sm_pool.tile([16, E * CH], BF, tag="offn", name="offn")
        offT = sm_pool.tile([128, 16, 16], BF, tag="offT", name="offT")
        qTs = [small.tile([128, 16], BF, tag="qT", name="qT") for _ in range(BLOC)]
        ctxT = small.tile([128, 4, BLOC], BF, tag="ctxT", name="ctxT")
        Zs = small.tile([40, 1], FP32, tag="Zs", name="Zs")
        Zb = small.tile([8, 2], FP32, tag="Zb", name="Zb")
        rzb = small.tile([8, 2], FP32, tag="rzb", name="rzb")
        Hs = [small.tile([8, 2, 128], BF, tag="Hs", name="Hs") for _ in range(BLOC)]
        heT = [small.tile([128, 2, 8], BF, tag="heT", name="heT") for _ in range(BLOC)]
        heoT = [small.tile([128, 2, 8], BF, tag="heoT", name="heoT") for _ in range(BLOC)]
        whTT = sm_pool.tile([40, CH], BF, tag="whTT", name="whTT")

        # memsets: garbage rows that feed the one-shot exp / xbar transposes
        # (gpsimd memset needs 32-aligned partition bases; valid rows are
        # overwritten by the logit drains / exp / offsets copy later)
        nc.gpsimd.memset(lgs[0:32, :], 0.0)
        nc.gpsimd.memset(Pn[32:48, :], 0.0)
        nc.gpsimd.memset(offn[:, :], 0.0)

        # ---------------- cv1 / cv2 ----------------
        def conv_chunks(b, wt, bias_s, out_tile, accum, hook=None):
            for m in range(2):
                for c2 in range(NCHUNKS):
                    ps = psum.tile([128, NCH], FP32, tag="big", name="big")
                    for kt in range(2):
                        for ns in range(NCH // NSUB):
                            nc.tensor.matmul(
                                ps[:, ns * NSUB:(ns + 1) * NSUB],
                                wt[:, kt, m * 128:(m + 1) * 128],
                                xs[b][:, kt, c2 * NCH + ns * NSUB: c2 * NCH + (ns + 1) * NSUB],
                                start=(kt == 0), stop=(kt == 1))
                    acc = tok_sums[b][:, m, c2:c2 + 1] if accum else None
                    nc.scalar.activation(
                        out_tile[:, m, c2 * NCH:(c2 + 1) * NCH], ps, AF.Silu,
                        bias=bias_s[:, m:m + 1], accum_out=acc)
                    if hook is not None:
                        hook(b, m, c2)

        TCH = NCH // 128  # 16 transposed t-blocks per chunk

        def cv1_hook(b, m, c2):
            nc.sync.dma_start(
                out=tl2[b][:, m * 32 + c2 * TCH: m * 32 + (c2 + 1) * TCH, :],
                in_=tokens[b][:, m, c2 * NCH:(c2 + 1) * NCH], transpose=True)
            nc.vector.reduce_max(maxp[b][:, m, c2:c2 + 1],
                                 tokens[b][:, m, c2 * NCH:(c2 + 1) * NCH], AX.X)

        for b in range(BLOC):
            conv_chunks(b, w1t, b1s, tokens[b], accum=True, hook=cv1_hook)

        # late weight loads, queued behind the first-needed ones
        gwt = ld_w("gwt", d["gwt"], 4, E * CH, nc.gpsimd)
        q0T = singles.tile([128, 16], BF, tag="q0T")
        nc.gpsimd.dma_start(out=q0T, in_=d["q0t"][:])
        ident = singles.tile([8, 8], BF, tag="ident")
        nc.gpsimd.dma_start(out=ident, in_=d["ident"][:])
        edgewt = ld_w("edgewt", d["edgewt"], 2, CH, nc.gpsimd)
        nodewt = ld_w("nodewt", d["nodewt"], 2, CH, nc.gpsimd)
        w3t = ld_w("w3t", d["w3t"], 4, C2, nc.gpsimd)
        b3s = ld_b("b3", d["b3"], nc.gpsimd)
        ebs, nbs = ld_b("eb", d["eb"], nc.gpsimd), ld_b("nb", d["nb"], nc.gpsimd)

        # cv2 batch 0 keeps ACT busy while the ctx chain resolves
        conv_chunks(0, w2t, b2s, y2[0], accum=False)

        # ---------------- ctx -> q-offsets (natural) -> transpose ---------
        for b in range(BLOC):
            avg_raw = small.tile([128, 2], FP32, tag="avgr", name="avgr")
            nc.vector.reduce_sum(avg_raw, tok_sums[b], AX.X)
            nc.vector.tensor_scalar_mul(ctxT[:, 0:2, b], avg_raw, 1.0 / N)
            for m in range(2):
                nc.vector.reduce_max(ctxT[:, 2 + m, b:b + 1], maxp[b][:, m, :], AX.X)

        # q-offsets natural [2, 2048]: stationary = ctxT (2 cols), moving = G
        ps_off = psum.tile([128, NCH], FP32, tag="big", name="big")
        for kt in range(4):
            for nb4 in range(4):
                nc.tensor.matmul(
                    ps_off[0:2, nb4 * NSUB:(nb4 + 1) * NSUB],
                    ctxT[:, kt, :],
                    gwt[:, kt, nb4 * NSUB:(nb4 + 1) * NSUB],
                    start=(kt == 0), stop=(kt == 3))
        nc.vector.tensor_copy(offn[0:2, :], ps_off[0:2, :])
        nc.sync.dma_start(out=offT, in_=offn, transpose=True)
        for b in range(BLOC):
            nc.vector.tensor_add(qTs[b], offT[:, :, b], q0T)

        # ---------------- logits (natural [e, n]) + DVE drains ------------
        for b in range(BLOC):
            for c2 in range(NCHUNKS):
                lp = psum.tile([128, NCH], FP32, tag="big", name="big")
                for kt in range(2):
                    for ns in range(NCH // NSUB):
                        nc.tensor.matmul(
                            lp[0:8, ns * NSUB:(ns + 1) * NSUB],
                            qTs[b][:, kt:16:2],
                            tokens[b][:, kt, c2 * NCH + ns * NSUB: c2 * NCH + (ns + 1) * NSUB],
                            start=(kt == 0), stop=(kt == 1))
                nc.vector.tensor_copy(
                    lgs[b * 32:b * 32 + 8, c2 * NCH:(c2 + 1) * NCH], lp[0:8, :])

        # cv2 batch 1: ACT work to hide the logits/softmax latency
        conv_chunks(1, w2t, b2s, y2[1], accum=False)

        # ---------------- softmax: one exp, Z via accumulator -------------
        nc.scalar.activation(Pn[0:40, :], lgs, AF.Exp, scale=LSCALE,
                             accum_out=Zs)
        nc.vector.tensor_copy(Zb[:, 0:1], Zs[0:8, :])
        nc.vector.tensor_copy(Zb[:, 1:2], Zs[32:40, :])
        nc.vector.reciprocal(rzb, Zb)
        nc.sync.dma_start(out=PT[:, :, 0:16], in_=Pn[0:16, :], transpose=True)
        nc.sync.dma_start(out=PT[:, :, 16:32], in_=Pn[32:48, :], transpose=True)

        # ---------------- He (natural) -> edge -> whTT --------------------
        for b in range(BLOC):
            hep = psum.tile([8, 2, 128], FP32, tag="big", name="hep")
            for t in range(32):
                nc.tensor.matmul(
                    hep,
                    PT[:, t, 16 * b:16 * b + 8],
                    tl2[b][:, t::32, :],
                    start=(t == 0), stop=(t == 31))
            # He normalized by 1/Z on the way out of PSUM
            nc.vector.tensor_scalar_mul(Hs[b], hep, rzb[:, b:b + 1])
            # He^T via PE transpose (out = in.T against identity)
            for m in range(2):
                tp = psum.tile([128, 8], BF, tag="big", name="tp")
                nc.tensor.transpose(tp, Hs[b][:, m, :], ident)
                nc.vector.tensor_copy(heT[b][:, m, :], tp)
            hop = psum.tile([128, 2, 8], FP32, tag="big", name="hop")
            for mq in range(2):
                for kt in range(2):
                    nc.tensor.matmul(
                        hop[:, mq, :],
                        edgewt[:, kt, mq * 128:(mq + 1) * 128],
                        heT[b][:, kt, :],
                        start=(kt == 0), stop=(kt == 1))
            for mq in range(2):
                nc.scalar.activation(heoT[b][:, mq, :], hop[:, mq, :], AF.Gelu,
                                     bias=ebs[:, mq:mq + 1])
            # whTT rows [32b, 32b+8) = (Heo @ node_w^T) * (1/Z)
            wp = psum.tile([8, CH], FP32, tag="big", name="wp")
            for kt in range(2):
                nc.tensor.matmul(
                    wp,
                    heoT[b][:, kt, :],
                    nodewt[:, kt, :],
                    start=(kt == 0), stop=(kt == 1))
            nc.vector.tensor_scalar_mul(whTT[b * 32:b * 32 + 8, :], wp,
                                        rzb[:, b:b + 1])

        # ---------------- node-apply + gelu; m_out = tokens + gelu --------
        m_out = [xs_pool.tile([128, 2, N], BF, tag="xs", name="xs") for _ in range(BLOC)]
        for b in range(BLOC):
            for m in range(2):
                for c2 in range(NCHUNKS):
                    ps = psum.tile([128, NCH], FP32, tag="big", name="big")
                    for ns in range(NCH // NSUB):
                        nc.tensor.matmul(
                            ps[:, ns * NSUB:(ns + 1) * NSUB],
                            whTT[b * 32:b * 32 + 8, m * 128:(m + 1) * 128],
                            Pn[b * 32:b * 32 + 8, c2 * NCH + ns * NSUB: c2 * NCH + (ns + 1) * NSUB],
                            start=True, stop=True,
                            tile_position=(b * 32, 0))
                    gel = stage.tile([128, NCH], BF, tag="stage", name="stage")
                    nc.scalar.activation(gel, ps, AF.Gelu, bias=nbs[:, m:m + 1])
                    nc.vector.tensor_add(m_out[b][:, m, c2 * NCH:(c2 + 1) * NCH],
                                         gel, tokens[b][:, m, c2 * NCH:(c2 + 1) * NCH])

        # ---------------- cv3 + SiLU + store ------------------------------
        for b in range(BLOC):
            for m in range(2):
                for c2 in range(NCHUNKS):
                    ps = psum.tile([128, NCH], FP32, tag="big", name="big")
                    for kt in range(4):
                        rhs_t = m_out[b] if kt < 2 else y2[b]
                        for ns in range(NCH // NSUB):
                            nc.tensor.matmul(
                                ps[:, ns * NSUB:(ns + 1) * NSUB],
                                w3t[:, kt, m * 128:(m + 1) * 128],
                                rhs_t[:, kt % 2, c2 * NCH + ns * NSUB: c2 * NCH + (ns + 1) * NSUB],
                                start=(kt == 0), stop=(kt == 3))
                    ostg = stage.tile([128, NCH], BF, tag="stage", name="stage")
                    nc.scalar.activation(ostg, ps, AF.Silu, bias=b3s[:, m:m + 1])
                    nc.gpsimd.dma_start(
                        out=d["out"][b, m * 128:(m + 1) * 128, c2 * NCH:(c2 + 1) * NCH],
                        in_=ostg)


@functools.cache
def get_nc():
    nc = bacc.Bacc("TRN2", target_bir_lowering=False, debug=False,
                   enable_asserts=False, num_devices=NCORES)
    emit_kernel(nc)
    nc.finalize()
    return nc


def prep_inputs(inputs):
    """Host-side weight folding + dtype casts. Returns per-core input maps."""
    f32 = np.float32

    def fold(w, g, b, m, v):
        s = (g / np.sqrt(v + EPS)).astype(f32)
        return (np.asarray(w, f32) * s[:, None]), (b - m * s).astype(f32)

    W1, b1 = fold(inputs["cv1_w"], inputs["cv1_g"], inputs["cv1_b"], inputs["cv1_m"], inputs["cv1_v"])
    W2, b2 = fold(inputs["cv2_w"], inputs["cv2_g"], inputs["cv2_b"], inputs["cv2_m"], inputs["cv2_v"])
    W3, b3 = fold(inputs["cv3_w"], inputs["cv3_g"], inputs["cv3_b"], inputs["cv3_m"], inputs["cv3_v"])
    proto_eff = np.asarray(inputs["proto"], f32) + np.asarray(inputs["ctx_b"], f32).reshape(E, CH)
    pre_w = np.asarray(inputs["pre_w"], f32)
    ctx_w = np.asarray(inputs["ctx_w"], f32)

    # q^T = q0T + G @ ctx  (pre_w folded into ctx_w and proto on host)
    # q0[e, c] = sum_c2 proto_eff[e, c2] * pre_w[c2, c]
    q0 = proto_eff @ pre_w
    q0T = q0.reshape(E, 2, 128).transpose(2, 0, 1).reshape(128, 16)
    # G[(e, c), k] = sum_c2 pre_w[c2, c] * ctx_w[e*CH + c2, k]
    cw3 = ctx_w.reshape(E, CH, 2 * CH)
    G = np.einsum("xc,exk->eck", pre_w, cw3).reshape(E * CH, 2 * CH)

    shared = {
        "w1t": np.ascontiguousarray(W1.T).astype(BF16),
        "w2t": np.ascontiguousarray(W2.T).astype(BF16),
        "w3t": np.ascontiguousarray(W3.T).astype(BF16),
        "gwt": np.ascontiguousarray(G.T).astype(BF16),
        "q0t": np.ascontiguousarray(q0T).astype(BF16),
        "ident": np.eye(8, dtype=f32).astype(BF16),
        "edgewt": np.ascontiguousarray(np.asarray(inputs["edge_w"], f32).T).astype(BF16),
        "nodewt": np.ascontiguousarray(np.asarray(inputs["node_w"], f32).T).astype(BF16),
        "b1": b1, "b2": b2, "b3": b3,
        "eb": np.asarray(inputs["edge_b"], f32),
        "nb": np.asarray(inputs["node_b"], f32),
    }
    x = np.asarray(inputs["x"], f32).reshape(B, C1, N).astype(BF16)
    in_maps = []
    for c in range(NCORES):
        m = dict(shared)
        m["x"] = np.ascontiguousarray(x[c * BLOC:(c + 1) * BLOC])
        in_maps.append(m)
    return in_maps


def run(inputs, trace=False, **kw):
    nc = get_nc()
    in_maps = prep_inputs(inputs)
    res = run_bass_kernel_spmd(nc, in_maps, list(range(NCORES)), trace=trace, **kw)
    outs = [np.asarray(res.results[i]["out"], np.float32) for i in range(NCORES)]
    full = np.concatenate(outs, axis=0).reshape(B, C2, H, W)
    return full, res


def kernel(**inputs):
    out, _ = run(inputs, trace=False)
    return out


# revision 6
# speedup vs baseline: 1.1875x; 1.1875x over previous
"""Trainium2 Bass kernel for nn_C3AH (C3-style hypergraph attention block).

Contract: kernel(**inputs) takes the FULL unsharded inputs (numpy f32) and
returns the FULL output [16, 256, 64, 64] f32.  Internally: data-parallel over
batch across 8 NeuronCores (2 batches per core), weights replicated, bf16
matmuls with f32 PSUM accumulation.

v3 — ACT-roofline rewrite.  Design notes (per core, 2 batches):
  - ACT (ScalarE) is the hard floor: 4 full-tensor activations
    (cv1/cv2 SiLU, node GELU, cv3 SiLU) = 8.4M elems @ 1.2GHz ~= 55us,
    + exp + 4 table loads.  Everything else is scheduled to hide under it.
  - Convs: [128, 2048] PSUM tiles (4 banks), kt-outer/ns-inner so the
    stationary operand is reused across 4 consecutive matmuls; one ACT call
    per 2048 chunk (amortizes the 352-cycle ACTIVATE overhead).
  - ctx offsets: pre_w is folded into ctx_w on the host (G = pre_w^T-app),
    so q^T = q0T + G-matmul(ctx).  The G matmul runs with ctx as the
    2-column stationary (LDWEIGHTS ~= 2 cols) and offsets come out natural
    [2, 2048]; a single xbar DMA transpose puts them back in [c-part, (e,m)]
    layout.  This kills the 64 LD-bound 128-col stationary loads the
    transposed-orientation matmul would need.
  - He aggregation flipped: stationary = P-block [128n, 8e] (8-col
    LDWEIGHTS ~= 7ns), moving = tokens^T blocks (both m-halves per matmul
    via a strided 3D AP).  He comes out natural [8, 256]; a PE-mode
    transpose (vs identity) flips it for the edge linear.
  - softmax: no-max-sub exp over [40, 4096] in ONE call with accum_out=Z;
    1/Z is folded into the He psum->sbuf copy and the whTT copy
    (per-partition tensor_scalar), so nothing normalizes the big A tensor.
  - node-apply packs b0/b1 in PE row-groups 0/32 (tile_position).
  - Engine balance: plain DMA on gpsimd (SWDGE), transposes on sync
    (HWDGE), PSUM drains on DVE.  ACT table sequence SILU->EXP->GELU->SILU
    (4 loads, minimal for this op sandwich).
"""
import sys
import functools

sys.path.insert(0, "/opt/trn_rl_repo")

import numpy as np
import ml_dtypes

import concourse.bass as bass
import concourse.tile as tile
from concourse import bacc, mybir
from concourse.bass_utils import run_bass_kernel_spmd

BF16 = ml_dtypes.bfloat16
FP32 = mybir.dt.float32
BF = mybir.dt.bfloat16
AF = mybir.ActivationFunctionType
AX = mybir.AxisListType

B, C1, H, W = 16, 256, 64, 64
N = H * W            # 4096
CH, C2, E = 256, 256, 8
NCORES = 8
BLOC = B // NCORES   # 2 batches per core
EPS = 1e-5
LSCALE = 1.0 / 64.0  # 1/(NH*sqrt(HD))

NCH = 2048           # free-dim chunk for conv PSUM tiles / ACT calls
NSUB = 512           # matmul moving-operand max (one PSUM bank fp32)
NCHUNKS = N // NCH   # 2


def emit_kernel(nc):
    # ---------------- DRAM I/O ----------------
    x_d = nc.dram_tensor("x", [BLOC, C1, N], BF, kind="ExternalInput")
    w1t_d = nc.dram_tensor("w1t", [C1, CH], BF, kind="ExternalInput")
    w2t_d = nc.dram_tensor("w2t", [C1, CH], BF, kind="ExternalInput")
    w3t_d = nc.dram_tensor("w3t", [2 * CH, C2], BF, kind="ExternalInput")
    gwt_d = nc.dram_tensor("gwt", [2 * CH, E * CH], BF, kind="ExternalInput")
    q0t_d = nc.dram_tensor("q0t", [128, 16], BF, kind="ExternalInput")
    ident_d = nc.dram_tensor("ident", [8, 8], BF, kind="ExternalInput")
    edgewt_d = nc.dram_tensor("edgewt", [CH, CH], BF, kind="ExternalInput")
    nodewt_d = nc.dram_tensor("nodewt", [CH, CH], BF, kind="ExternalInput")
    b1_d = nc.dram_tensor("b1", [CH], FP32, kind="ExternalInput")
    b2_d = nc.dram_tensor("b2", [CH], FP32, kind="ExternalInput")
    b3_d = nc.dram_tensor("b3", [C2], FP32, kind="ExternalInput")
    eb_d = nc.dram_tensor("eb", [CH], FP32, kind="ExternalInput")
    nb_d = nc.dram_tensor("nb", [CH], FP32, kind="ExternalInput")
    out_d = nc.dram_tensor("out", [BLOC, C2, N], BF, kind="ExternalOutput")

    with tile.TileContext(nc) as tc:
        emit_body(nc, tc, dict(
            x=x_d, w1t=w1t_d, w2t=w2t_d, w3t=w3t_d, gwt=gwt_d, q0t=q0t_d,
            ident=ident_d, edgewt=edgewt_d, nodewt=nodewt_d,
            b1=b1_d, b2=b2_d, b3=b3_d, eb=eb_d, nb=nb_d, out=out_d))
    return nc


def emit_body(nc, tc, d):
    from contextlib import ExitStack
    ctx = ExitStack()
    with ctx:
        singles = ctx.enter_context(tc.tile_pool(name="singles", bufs=1))
        xs_pool = ctx.enter_context(tc.tile_pool(name="xs", bufs=2))
        tok_pool = ctx.enter_context(tc.tile_pool(name="tok", bufs=2))
        y2_pool = ctx.enter_context(tc.tile_pool(name="y2", bufs=2))
        l2_pool = ctx.enter_context(tc.tile_pool(name="l2", bufs=2))
        sm_pool = ctx.enter_context(tc.tile_pool(name="sm", bufs=1))
        small = ctx.enter_context(tc.tile_pool(name="small", bufs=2))
        stage = ctx.enter_context(tc.tile_pool(name="stage", bufs=3))
        # PSUM: 2 x [128, 2048] f32 = 2 x 4 banks = all 8 banks.  Small
        # attention-phase tiles rotate through the same ring.
        psum = ctx.enter_context(tc.tile_pool(name="psum", bufs=2, space="PSUM"))

        # ---------------- loads ----------------
        def ld_w(name, dram, kt, mcols, eng):
            t = singles.tile([128, kt, mcols], BF, tag=name)
            eng.dma_start(out=t, in_=dram[:].rearrange("(t p) m -> p t m", p=128))
            return t

        def ld_b(name, dram, eng):
            t = singles.tile([128, 2], FP32, tag=name)
            eng.dma_start(out=t, in_=dram[:].rearrange("(t p) -> p t", p=128))
            return t

        w1t = ld_w("w1t", d["w1t"], 2, CH, nc.gpsimd)
        b1s = ld_b("b1", d["b1"], nc.gpsimd)

        xs = [xs_pool.tile([128, 2, N], BF, tag="xs", name="xs") for _ in range(BLOC)]
        for b in range(BLOC):
            xr = d["x"][b].rearrange("(t p) n -> p t n", p=128)
            for c2 in range(NCHUNKS):
                nc.sync.dma_start(out=xs[b][:, :, c2 * NCH:(c2 + 1) * NCH],
                                  in_=xr[:, :, c2 * NCH:(c2 + 1) * NCH])

        w2t = ld_w("w2t", d["w2t"], 2, CH, nc.gpsimd)
        b2s = ld_b("b2", d["b2"], nc.gpsimd)

        # persistent activation-side tiles
        tokens = [tok_pool.tile([128, 2, N], BF, tag="tok", name="tok") for _ in range(BLOC)]
        y2 = [y2_pool.tile([128, 2, N], BF, tag="y2", name="y2") for _ in range(BLOC)]
        # tokens^T per batch: tl2[b][:, m*32+t, :] = tokens[b][:, m, 128t:128t+128]^T
        tl2 = [l2_pool.tile([128, 64, 128], BF, tag="l2", name="l2") for _ in range(BLOC)]
        tok_sums = [small.tile([128, 2, NCHUNKS], FP32, tag="tsum", name="tsum") for _ in range(BLOC)]
        # bf16 max partials: exact (inputs are bf16) and 2x DVE rate
        maxp = [small.tile([128, 2, NCHUNKS], BF, tag="maxp", name="maxp") for _ in range(BLOC)]

        # softmax / attention state (batch b at partition rows [32b, 32b+8))
        lgs = sm_pool.tile([40, N], BF, tag="lgs", name="lgs")
        Pn = sm_pool.tile([48, N], BF, tag="Pn", name="Pn")
        PT = sm_pool.tile([128, 32, 32], BF, tag="PT", name="PT")
        offn = sm_pool.tile([16, E * CH], BF, tag="offn", name="offn")
        offT = sm_pool.tile([128, 16, 16], BF, tag="offT", name="offT")
        qTs = [small.tile([128, 16], BF, tag="qT", name="qT") for _ in range(BLOC)]
        ctxT = small.tile([128, 4, BLOC], BF, tag="ctxT", name="ctxT")
        Zs = small.tile([40, 1], FP32, tag="Zs", name="Zs")
        Zb = small.tile([8, 2], FP32, tag="Zb", name="Zb")
        rzb = small.tile([8, 2], FP32, tag="rzb", name="rzb")
        Hs = [small.tile([8, 2, 128], BF, tag="Hs", name="Hs") for _ in range(BLOC)]
        heT = [small.tile([128, 2, 8], BF, tag="heT", name="heT") for _ in range(BLOC)]
        heoT = [small.tile([128, 2, 8], BF, tag="heoT", name="heoT") for _ in range(BLOC)]
        whTT = sm_pool.tile([40, CH], BF, tag="whTT", name="whTT")

        # memsets: garbage rows that feed the one-shot exp / xbar transposes
        # (gpsimd memset needs 32-aligned partition bases; valid rows are
        # overwritten by the logit drains / exp / offsets copy later)
        nc.gpsimd.memset(lgs[0:32, :], 0.0)
        nc.gpsimd.memset(Pn[32:48, :], 0.0)
        nc.gpsimd.memset(offn[:, :], 0.0)

        # ---------------- cv1 / cv2 ----------------
        def conv_chunks(b, wt, bias_s, out_tile, accum, hook=None):
            for m in range(2):
                for c2 in range(NCHUNKS):
                    ps = psum.tile([128, NCH], FP32, tag="big", name="big")
                    for kt in range(2):
                        for ns in range(NCH // NSUB):
                            nc.tensor.matmul(
                                ps[:, ns * NSUB:(ns + 1) * NSUB],
                                wt[:, kt, m * 128:(m + 1) * 128],
                                xs[b][:, kt, c2 * NCH + ns * NSUB: c2 * NCH + (ns + 1) * NSUB],
                                start=(kt == 0), stop=(kt == 1))
                    acc = tok_sums[b][:, m, c2:c2 + 1] if accum else None
                    nc.scalar.activation(
                        out_tile[:, m, c2 * NCH:(c2 + 1) * NCH], ps, AF.Silu,
                        bias=bias_s[:, m:m + 1], accum_out=acc)
                    if hook is not None:
                        hook(b, m, c2)

        TCH = NCH // 128  # 16 transposed t-blocks per chunk

        def cv1_hook(b, m, c2):
            nc.sync.dma_start(
                out=tl2[b][:, m * 32 + c2 * TCH: m * 32 + (c2 + 1) * TCH, :],
                in_=tokens[b][:, m, c2 * NCH:(c2 + 1) * NCH], transpose=True)
            nc.vector.reduce_max(maxp[b][:, m, c2:c2 + 1],
                                 tokens[b][:, m, c2 * NCH:(c2 + 1) * NCH], AX.X)

        for b in range(BLOC):
            conv_chunks(b, w1t, b1s, tokens[b], accum=True, hook=cv1_hook)

        # late weight loads, queued behind the first-needed ones
        gwt = ld_w("gwt", d["gwt"], 4, E * CH, nc.gpsimd)
        q0T = singles.tile([128, 16], BF, tag="q0T")
        nc.gpsimd.dma_start(out=q0T, in_=d["q0t"][:])
        ident = singles.tile([8, 8], BF, tag="ident")
        nc.gpsimd.dma_start(out=ident, in_=d["ident"][:])
        edgewt = ld_w("edgewt", d["edgewt"], 2, CH, nc.gpsimd)
        nodewt = ld_w("nodewt", d["nodewt"], 2, CH, nc.gpsimd)
        w3t = ld_w("w3t", d["w3t"], 4, C2, nc.gpsimd)
        b3s = ld_b("b3", d["b3"], nc.gpsimd)
        ebs, nbs = ld_b("eb", d["eb"], nc.gpsimd), ld_b("nb", d["nb"], nc.gpsimd)

        # cv2 batch 0 keeps ACT busy while the ctx chain resolves
        conv_chunks(0, w2t, b2s, y2[0], accum=False)

        # ---------------- ctx -> q-offsets (natural) -> transpose ---------
        for b in range(BLOC):
            avg_raw = small.tile([128, 2], FP32, tag="avgr", name="avgr")
            nc.vector.reduce_sum(avg_raw, tok_sums[b], AX.X)
            nc.vector.tensor_scalar_mul(ctxT[:, 0:2, b], avg_raw, 1.0 / N)
            for m in range(2):
                nc.vector.reduce_max(ctxT[:, 2 + m, b:b + 1], maxp[b][:, m, :], AX.X)

        # q-offsets natural [2, 2048]: stationary = ctxT (2 cols), moving = G
        ps_off = psum.tile([128, NCH], FP32, tag="big", name="big")
        for kt in range(4):
            for nb4 in range(4):
                nc.tensor.matmul(
                    ps_off[0:2, nb4 * NSUB:(nb4 + 1) * NSUB],
                    ctxT[:, kt, :],
                    gwt[:, kt, nb4 * NSUB:(nb4 + 1) * NSUB],
                    start=(kt == 0), stop=(kt == 3))
        nc.vector.tensor_copy(offn[0:2, :], ps_off[0:2, :])
        # HWDGE transpose issued from ScalarE: lands between cv2 SILU calls
        # in the ACT FIFO, so it doesn't queue behind the tl2 transposes on
        # sync (head-of-line blocking of the whole attention chain).
        nc.scalar.dma_start(out=offT, in_=offn, transpose=True)
        for b in range(BLOC):
            nc.vector.tensor_add(qTs[b], offT[:, :, b], q0T)

        # cv2 batch 1: PE + ACT work to hide the ctx/logits latency
        conv_chunks(1, w2t, b2s, y2[1], accum=False)

        # ---------------- logits (natural [e, n]) + DVE drains ------------
        for b in range(BLOC):
            for c2 in range(NCHUNKS):
                lp = psum.tile([128, NCH], FP32, tag="big", name="big")
                for kt in range(2):
                    for ns in range(NCH // NSUB):
                        nc.tensor.matmul(
                            lp[0:8, ns * NSUB:(ns + 1) * NSUB],
                            qTs[b][:, kt:16:2],
                            tokens[b][:, kt, c2 * NCH + ns * NSUB: c2 * NCH + (ns + 1) * NSUB],
                            start=(kt == 0), stop=(kt == 1))
                nc.vector.tensor_copy(
                    lgs[b * 32:b * 32 + 8, c2 * NCH:(c2 + 1) * NCH], lp[0:8, :])

        # ---------------- softmax: one exp, Z via accumulator -------------
        nc.scalar.activation(Pn[0:40, :], lgs, AF.Exp, scale=LSCALE,
                             accum_out=Zs)
        nc.vector.tensor_copy(Zb[:, 0:1], Zs[0:8, :])
        nc.vector.tensor_copy(Zb[:, 1:2], Zs[32:40, :])
        nc.vector.reciprocal(rzb, Zb)
        nc.scalar.dma_start(out=PT[:, :, 0:16], in_=Pn[0:16, :], transpose=True)
        nc.scalar.dma_start(out=PT[:, :, 16:32], in_=Pn[32:48, :], transpose=True)

        # ---------------- He (natural) -> edge -> whTT --------------------
        for b in range(BLOC):
            hep = psum.tile([8, 2, 128], FP32, tag="big", name="hep")
            for t in range(32):
                nc.tensor.matmul(
                    hep,
                    PT[:, t, 16 * b:16 * b + 8],
                    tl2[b][:, t::32, :],
                    start=(t == 0), stop=(t == 31))
            # He normalized by 1/Z on the way out of PSUM
            nc.vector.tensor_scalar_mul(Hs[b], hep, rzb[:, b:b + 1])
            # He^T via PE transpose (out = in.T against identity)
            for m in range(2):
                tp = psum.tile([128, 8], BF, tag="big", name="tp")
                nc.tensor.transpose(tp, Hs[b][:, m, :], ident)
                nc.vector.tensor_copy(heT[b][:, m, :], tp)
            hop = psum.tile([128, 2, 8], FP32, tag="big", name="hop")
            for mq in range(2):
                for kt in range(2):
                    nc.tensor.matmul(
                        hop[:, mq, :],
                        edgewt[:, kt, mq * 128:(mq + 1) * 128],
                        heT[b][:, kt, :],
                        start=(kt == 0), stop=(kt == 1))
            for mq in range(2):
                nc.scalar.activation(heoT[b][:, mq, :], hop[:, mq, :], AF.Gelu,
                                     bias=ebs[:, mq:mq + 1])
            # whTT rows [32b, 32b+8) = (Heo @ node_w^T) * (1/Z)
            wp = psum.tile([8, CH], FP32, tag="big", name="wp")
            for kt in range(2):
                nc.tensor.matmul(
                    wp,
                    heoT[b][:, kt, :],
                    nodewt[:, kt, :],
                    start=(kt == 0), stop=(kt == 1))
            nc.vector.tensor_scalar_mul(whTT[b * 32:b * 32 + 8, :], wp,
                                        rzb[:, b:b + 1])

        # ---------------- node-apply + gelu; m_out = tokens + gelu --------
        m_out = [xs_pool.tile([128, 2, N], BF, tag="xs", name="xs") for _ in range(BLOC)]
        for b in range(BLOC):
            for m in range(2):
                for c2 in range(NCHUNKS):
                    ps = psum.tile([128, NCH], FP32, tag="big", name="big")
                    for ns in range(NCH // NSUB):
                        nc.tensor.matmul(
                            ps[:, ns * NSUB:(ns + 1) * NSUB],
                            whTT[b * 32:b * 32 + 8, m * 128:(m + 1) * 128],
                            Pn[b * 32:b * 32 + 8, c2 * NCH + ns * NSUB: c2 * NCH + (ns + 1) * NSUB],
                            start=True, stop=True,
                            tile_position=(b * 32, 0))
                    gel = stage.tile([128, NCH], BF, tag="stage", name="stage")
                    nc.scalar.activation(gel, ps, AF.Gelu, bias=nbs[:, m:m + 1])
                    nc.vector.tensor_add(m_out[b][:, m, c2 * NCH:(c2 + 1) * NCH],
                                         gel, tokens[b][:, m, c2 * NCH:(c2 + 1) * NCH])

        # ---------------- cv3 + SiLU + store ------------------------------
        for b in range(BLOC):
            for m in range(2):
                for c2 in range(NCHUNKS):
                    ps = psum.tile([128, NCH], FP32, tag="big", name="big")
                    for kt in range(4):
                        rhs_t = m_out[b] if kt < 2 else y2[b]
                        for ns in range(NCH // NSUB):
                            nc.tensor.matmul(
                                ps[:, ns * NSUB:(ns + 1) * NSUB],
                                w3t[:, kt, m * 128:(m + 1) * 128],
                                rhs_t[:, kt % 2, c2 * NCH + ns * NSUB: c2 * NCH + (ns + 1) * NSUB],
                                start=(kt == 0), stop=(kt == 3))
                    ostg = stage.tile([128, NCH], BF, tag="stage", name="stage")
                    nc.scalar.activation(ostg, ps, AF.Silu, bias=b3s[:, m:m + 1])
                    nc.gpsimd.dma_start(
                        out=d["out"][b, m * 128:(m + 1) * 128, c2 * NCH:(c2 + 1) * NCH],
                        in_=ostg)


@functools.cache
def get_nc():
    nc = bacc.Bacc("TRN2", target_bir_lowering=False, debug=False,
                   enable_asserts=False, num_devices=NCORES)
    emit_kernel(nc)
    nc.finalize()
    return nc


def prep_inputs(inputs):
    """Host-side weight folding + dtype casts. Returns per-core input maps."""
    f32 = np.float32

    def fold(w, g, b, m, v):
        s = (g / np.sqrt(v + EPS)).astype(f32)
        return (np.asarray(w, f32) * s[:, None]), (b - m * s).astype(f32)

    W1, b1 = fold(inputs["cv1_w"], inputs["cv1_g"], inputs["cv1_b"], inputs["cv1_m"], inputs["cv1_v"])
    W2, b2 = fold(inputs["cv2_w"], inputs["cv2_g"], inputs["cv2_b"], inputs["cv2_m"], inputs["cv2_v"])
    W3, b3 = fold(inputs["cv3_w"], inputs["cv3_g"], inputs["cv3_b"], inputs["cv3_m"], inputs["cv3_v"])
    proto_eff = np.asarray(inputs["proto"], f32) + np.asarray(inputs["ctx_b"], f32).reshape(E, CH)
    pre_w = np.asarray(inputs["pre_w"], f32)
    ctx_w = np.asarray(inputs["ctx_w"], f32)

    # q^T = q0T + G @ ctx  (pre_w folded into ctx_w and proto on host)
    # q0[e, c] = sum_c2 proto_eff[e, c2] * pre_w[c2, c]
    q0 = proto_eff @ pre_w
    q0T = q0.reshape(E, 2, 128).transpose(2, 0, 1).reshape(128, 16)
    # G[(e, c), k] = sum_c2 pre_w[c2, c] * ctx_w[e*CH + c2, k]
    cw3 = ctx_w.reshape(E, CH, 2 * CH)
    G = np.einsum("xc,exk->eck", pre_w, cw3).reshape(E * CH, 2 * CH)

    shared = {
        "w1t": np.ascontiguousarray(W1.T).astype(BF16),
        "w2t": np.ascontiguousarray(W2.T).astype(BF16),
        "w3t": np.ascontiguousarray(W3.T).astype(BF16),
        "gwt": np.ascontiguousarray(G.T).astype(BF16),
        "q0t": np.ascontiguousarray(q0T).astype(BF16),
        "ident": np.eye(8, dtype=f32).astype(BF16),
        "edgewt": np.ascontiguousarray(np.asarray(inputs["edge_w"], f32).T).astype(BF16),
        "nodewt": np.ascontiguousarray(np.asarray(inputs["node_w"], f32).T).astype(BF16),
        "b1": b1, "b2": b2, "b3": b3,
        "eb": np.asarray(inputs["edge_b"], f32),
        "nb": np.asarray(inputs["node_b"], f32),
    }
    x = np.asarray(inputs["x"], f32).reshape(B, C1, N).astype(BF16)
    in_maps = []
    for c in range(NCORES):
        m = dict(shared)
        m["x"] = np.ascontiguousarray(x[c * BLOC:(c + 1) * BLOC])
        in_maps.append(m)
    return in_maps


def run(inputs, trace=False, **kw):
    nc = get_nc()
    in_maps = prep_inputs(inputs)
    res = run_bass_kernel_spmd(nc, in_maps, list(range(NCORES)), trace=trace, **kw)
    outs = [np.asarray(res.results[i]["out"], np.float32) for i in range(NCORES)]
    full = np.concatenate(outs, axis=0).reshape(B, C2, H, W)
    return full, res


def kernel(**inputs):
    out, _ = run(inputs, trace=False)
    return out


# revision 12
# speedup vs baseline: 1.1889x; 1.0012x over previous
"""Trainium2 Bass kernel for nn_C3AH (C3-style hypergraph attention block).

Contract: kernel(**inputs) takes the FULL unsharded inputs (numpy f32) and
returns the FULL output [16, 256, 64, 64] f32.  Internally: data-parallel over
batch across 8 NeuronCores (2 batches per core), weights replicated, bf16
matmuls with f32 PSUM accumulation.

v3 — ACT-roofline rewrite.  Design notes (per core, 2 batches):
  - ACT (ScalarE) is the hard floor: 4 full-tensor activations
    (cv1/cv2 SiLU, node GELU, cv3 SiLU) = 8.4M elems @ 1.2GHz ~= 55us,
    + exp + 4 table loads.  Everything else is scheduled to hide under it.
  - Convs: [128, 2048] PSUM tiles (4 banks), kt-outer/ns-inner so the
    stationary operand is reused across 4 consecutive matmuls; one ACT call
    per 2048 chunk (amortizes the 352-cycle ACTIVATE overhead).
  - ctx offsets: pre_w is folded into ctx_w on the host (G = pre_w^T-app),
    so q^T = q0T + G-matmul(ctx).  The G matmul runs with ctx as the
    2-column stationary (LDWEIGHTS ~= 2 cols) and offsets come out natural
    [2, 2048]; a single xbar DMA transpose puts them back in [c-part, (e,m)]
    layout.  This kills the 64 LD-bound 128-col stationary loads the
    transposed-orientation matmul would need.
  - He aggregation flipped: stationary = P-block [128n, 8e] (8-col
    LDWEIGHTS ~= 7ns), moving = tokens^T blocks (both m-halves per matmul
    via a strided 3D AP).  He comes out natural [8, 256]; a PE-mode
    transpose (vs identity) flips it for the edge linear.
  - softmax: no-max-sub exp over [40, 4096] in ONE call with accum_out=Z;
    1/Z is folded into the He psum->sbuf copy and the whTT copy
    (per-partition tensor_scalar), so nothing normalizes the big A tensor.
  - node-apply packs b0/b1 in PE row-groups 0/32 (tile_position).
  - Engine balance: plain DMA on gpsimd (SWDGE), transposes on sync
    (HWDGE), PSUM drains on DVE.  ACT table sequence SILU->EXP->GELU->SILU
    (4 loads, minimal for this op sandwich).
"""
import sys
import functools

sys.path.insert(0, "/opt/trn_rl_repo")

import numpy as np
import ml_dtypes

import concourse.bass as bass
import concourse.tile as tile
from concourse import bacc, mybir
from concourse.bass_utils import run_bass_kernel_spmd

BF16 = ml_dtypes.bfloat16
FP32 = mybir.dt.float32
BF = mybir.dt.bfloat16
AF = mybir.ActivationFunctionType
AX = mybir.AxisListType

B, C1, H, W = 16, 256, 64, 64
N = H * W            # 4096
CH, C2, E = 256, 256, 8
NCORES = 8
BLOC = B // NCORES   # 2 batches per core
EPS = 1e-5
LSCALE = 1.0 / 64.0  # 1/(NH*sqrt(HD))

NCH = 2048           # free-dim chunk for conv PSUM tiles / ACT calls
NSUB = 512           # matmul moving-operand max (one PSUM bank fp32)
NCHUNKS = N // NCH   # 2


def emit_kernel(nc):
    # ---------------- DRAM I/O ----------------
    x_d = nc.dram_tensor("x", [BLOC, C1, N], BF, kind="ExternalInput")
    w1t_d = nc.dram_tensor("w1t", [C1, CH], BF, kind="ExternalInput")
    w2t_d = nc.dram_tensor("w2t", [C1, CH], BF, kind="ExternalInput")
    w3t_d = nc.dram_tensor("w3t", [2 * CH, C2], BF, kind="ExternalInput")
    gwt_d = nc.dram_tensor("gwt", [2 * CH, E * CH], BF, kind="ExternalInput")
    q0t_d = nc.dram_tensor("q0t", [128, 16], BF, kind="ExternalInput")
    ident_d = nc.dram_tensor("ident", [8, 8], BF, kind="ExternalInput")
    edgewt_d = nc.dram_tensor("edgewt", [CH, CH], BF, kind="ExternalInput")
    nodewt_d = nc.dram_tensor("nodewt", [CH, CH], BF, kind="ExternalInput")
    b1_d = nc.dram_tensor("b1", [CH], FP32, kind="ExternalInput")
    b2_d = nc.dram_tensor("b2", [CH], FP32, kind="ExternalInput")
    b3_d = nc.dram_tensor("b3", [C2], FP32, kind="ExternalInput")
    eb_d = nc.dram_tensor("eb", [CH], FP32, kind="ExternalInput")
    nb_d = nc.dram_tensor("nb", [CH], FP32, kind="ExternalInput")
    out_d = nc.dram_tensor("out", [BLOC, C2, N], BF, kind="ExternalOutput")

    with tile.TileContext(nc) as tc:
        emit_body(nc, tc, dict(
            x=x_d, w1t=w1t_d, w2t=w2t_d, w3t=w3t_d, gwt=gwt_d, q0t=q0t_d,
            ident=ident_d, edgewt=edgewt_d, nodewt=nodewt_d,
            b1=b1_d, b2=b2_d, b3=b3_d, eb=eb_d, nb=nb_d, out=out_d))
    return nc


def emit_body(nc, tc, d):
    from contextlib import ExitStack
    ctx = ExitStack()
    with ctx:
        singles = ctx.enter_context(tc.tile_pool(name="singles", bufs=1))
        xs_pool = ctx.enter_context(tc.tile_pool(name="xs", bufs=2))
        tok_pool = ctx.enter_context(tc.tile_pool(name="tok", bufs=2))
        y2_pool = ctx.enter_context(tc.tile_pool(name="y2", bufs=2))
        l2_pool = ctx.enter_context(tc.tile_pool(name="l2", bufs=2))
        sm_pool = ctx.enter_context(tc.tile_pool(name="sm", bufs=1))
        small = ctx.enter_context(tc.tile_pool(name="small", bufs=2))
        stage = ctx.enter_context(tc.tile_pool(name="stage", bufs=3))
        # PSUM: 2 x [128, 2048] f32 = 2 x 4 banks = all 8 banks.  Small
        # attention-phase tiles rotate through the same ring.
        psum = ctx.enter_context(tc.tile_pool(name="psum", bufs=2, space="PSUM"))

        # ---------------- loads ----------------
        def ld_w(name, dram, kt, mcols, eng):
            t = singles.tile([128, kt, mcols], BF, tag=name)
            eng.dma_start(out=t, in_=dram[:].rearrange("(t p) m -> p t m", p=128))
            return t

        def ld_b(name, dram, eng):
            t = singles.tile([128, 2], FP32, tag=name)
            eng.dma_start(out=t, in_=dram[:].rearrange("(t p) -> p t", p=128))
            return t

        w1t = ld_w("w1t", d["w1t"], 2, CH, nc.gpsimd)
        b1s = ld_b("b1", d["b1"], nc.gpsimd)

        xs = [xs_pool.tile([128, 2, N], BF, tag="xs", name="xs") for _ in range(BLOC)]
        XCH = 1024  # small load chunks so the first conv matmul starts early
        for b in range(BLOC):
            xr = d["x"][b].rearrange("(t p) n -> p t n", p=128)
            for xc in range(N // XCH):
                nc.sync.dma_start(out=xs[b][:, :, xc * XCH:(xc + 1) * XCH],
                                  in_=xr[:, :, xc * XCH:(xc + 1) * XCH])

        w2t = ld_w("w2t", d["w2t"], 2, CH, nc.gpsimd)
        b2s = ld_b("b2", d["b2"], nc.gpsimd)

        # persistent activation-side tiles
        tokens = [tok_pool.tile([128, 2, N], BF, tag="tok", name="tok") for _ in range(BLOC)]
        y2 = [y2_pool.tile([128, 2, N], BF, tag="y2", name="y2") for _ in range(BLOC)]
        # tokens^T per batch: tl2[b][:, m*32+t, :] = tokens[b][:, m, 128t:128t+128]^T
        tl2 = [l2_pool.tile([128, 64, 128], BF, tag="l2", name="l2") for _ in range(BLOC)]
        tok_sums = [small.tile([128, 2, NCHUNKS], FP32, tag="tsum", name="tsum") for _ in range(BLOC)]
        # bf16 max partials: exact (inputs are bf16) and 2x DVE rate
        maxp = [small.tile([128, 2, NCHUNKS], BF, tag="maxp", name="maxp") for _ in range(BLOC)]

        # softmax / attention state (batch b at partition rows [32b, 32b+8))
        Pn = sm_pool.tile([48, N], BF, tag="Pn", name="Pn")
        PT = sm_pool.tile([128, 32, 32], BF, tag="PT", name="PT")
        offn = sm_pool.tile([16, E * CH], BF, tag="offn", name="offn")
        offT = sm_pool.tile([128, 16, 16], BF, tag="offT", name="offT")
        qTs = [small.tile([128, 16], BF, tag="qT", name="qT") for _ in range(BLOC)]
        ctxT = small.tile([128, 4, BLOC], BF, tag="ctxT", name="ctxT")
        Zs4 = small.tile([40, 2], FP32, tag="Zs4", name="Zs4")
        Zb = small.tile([8, 2], FP32, tag="Zb", name="Zb")
        rzb = small.tile([8, 2], FP32, tag="rzb", name="rzb")
        Hs = [small.tile([8, 2, 128], BF, tag="Hs", name="Hs") for _ in range(BLOC)]
        heT = [small.tile([128, 2, 8], BF, tag="heT", name="heT") for _ in range(BLOC)]
        heoT = [small.tile([128, 2, 8], BF, tag="heoT", name="heoT") for _ in range(BLOC)]
        whTT = sm_pool.tile([40, CH], BF, tag="whTT", name="whTT")

        # memsets: garbage rows that feed the xbar transposes must be
        # initialized (gpsimd memset needs 32-aligned partition bases; the
        # valid rows are overwritten by exp / the offsets copy later)
        nc.gpsimd.memset(Pn[0:32, :], 0.0)
        nc.gpsimd.memset(Pn[32:48, :], 0.0)
        nc.gpsimd.memset(offn[:, :], 0.0)

        # ---------------- cv1 / cv2 ----------------
        def conv_chunks(b, wt, bias_s, out_tile, accum, hook=None):
            for m in range(2):
                for c2 in range(NCHUNKS):
                    ps = psum.tile([128, NCH], FP32, tag="big", name="big")
                    for kt in range(2):
                        for ns in range(NCH // NSUB):
                            nc.tensor.matmul(
                                ps[:, ns * NSUB:(ns + 1) * NSUB],
                                wt[:, kt, m * 128:(m + 1) * 128],
                                xs[b][:, kt, c2 * NCH + ns * NSUB: c2 * NCH + (ns + 1) * NSUB],
                                start=(kt == 0), stop=(kt == 1))
                    acc = tok_sums[b][:, m, c2:c2 + 1] if accum else None
                    nc.scalar.activation(
                        out_tile[:, m, c2 * NCH:(c2 + 1) * NCH], ps, AF.Silu,
                        bias=bias_s[:, m:m + 1], accum_out=acc)
                    if hook is not None:
                        hook(b, m, c2)

        TCH = NCH // 128  # 16 transposed t-blocks per chunk

        def cv1_hook(b, m, c2):
            nc.sync.dma_start(
                out=tl2[b][:, m * 32 + c2 * TCH: m * 32 + (c2 + 1) * TCH, :],
                in_=tokens[b][:, m, c2 * NCH:(c2 + 1) * NCH], transpose=True)
            nc.vector.reduce_max(maxp[b][:, m, c2:c2 + 1],
                                 tokens[b][:, m, c2 * NCH:(c2 + 1) * NCH], AX.X)

        for b in range(BLOC):
            conv_chunks(b, w1t, b1s, tokens[b], accum=True, hook=cv1_hook)

        # late weight loads, queued behind the first-needed ones
        gwt = ld_w("gwt", d["gwt"], 4, E * CH, nc.gpsimd)
        q0T = singles.tile([128, 16], BF, tag="q0T")
        nc.gpsimd.dma_start(out=q0T, in_=d["q0t"][:])
        ident = singles.tile([8, 8], BF, tag="ident")
        nc.gpsimd.dma_start(out=ident, in_=d["ident"][:])
        edgewt = ld_w("edgewt", d["edgewt"], 2, CH, nc.gpsimd)
        nodewt = ld_w("nodewt", d["nodewt"], 2, CH, nc.gpsimd)
        w3t = ld_w("w3t", d["w3t"], 4, C2, nc.gpsimd)
        b3s = ld_b("b3", d["b3"], nc.gpsimd)
        ebs, nbs = ld_b("eb", d["eb"], nc.gpsimd), ld_b("nb", d["nb"], nc.gpsimd)

        # cv2 batch 0 keeps ACT busy while the ctx chain resolves
        conv_chunks(0, w2t, b2s, y2[0], accum=False)

        # ---------------- ctx -> q-offsets (natural) -> transpose ---------
        for b in range(BLOC):
            avg_raw = small.tile([128, 2], FP32, tag="avgr", name="avgr")
            nc.vector.reduce_sum(avg_raw, tok_sums[b], AX.X)
            nc.vector.tensor_scalar_mul(ctxT[:, 0:2, b], avg_raw, 1.0 / N)
            for m in range(2):
                nc.vector.reduce_max(ctxT[:, 2 + m, b:b + 1], maxp[b][:, m, :], AX.X)

        # q-offsets natural [2, 2048]: stationary = ctxT (2 cols), moving = G
        ps_off = psum.tile([128, NCH], FP32, tag="big", name="big")
        for kt in range(4):
            for nb4 in range(4):
                nc.tensor.matmul(
                    ps_off[0:2, nb4 * NSUB:(nb4 + 1) * NSUB],
                    ctxT[:, kt, :],
                    gwt[:, kt, nb4 * NSUB:(nb4 + 1) * NSUB],
                    start=(kt == 0), stop=(kt == 3))
        nc.vector.tensor_copy(offn[0:2, :], ps_off[0:2, :])
        # HWDGE transpose issued from ScalarE: lands between cv2 SILU calls
        # in the ACT FIFO, so it doesn't queue behind the tl2 transposes on
        # sync (head-of-line blocking of the whole attention chain).
        nc.scalar.dma_start(out=offT, in_=offn, transpose=True)
        for b in range(BLOC):
            nc.vector.tensor_add(qTs[b], offT[:, :, b], q0T)

        # ---------------- logits (natural [e, n]) + exp straight from PSUM
        # batch b's logits land at psum rows [32b, 32b+8) (col tile_position)
        # so exp reads/writes the same partition base; Z accumulates per call.
        for b in range(BLOC):
            for c2 in range(NCHUNKS):
                lp = psum.tile([128, NCH], FP32, tag="big", name="big")
                r = b * 32
                for kt in range(2):
                    for ns in range(NCH // NSUB):
                        nc.tensor.matmul(
                            lp[r:r + 8, ns * NSUB:(ns + 1) * NSUB],
                            qTs[b][:, kt:16:2],
                            tokens[b][:, kt, c2 * NCH + ns * NSUB: c2 * NCH + (ns + 1) * NSUB],
                            start=(kt == 0), stop=(kt == 1),
                            tile_position=(0, r))
                nc.scalar.activation(
                    Pn[r:r + 8, c2 * NCH:(c2 + 1) * NCH],
                    lp[r:r + 8, :], AF.Exp, scale=LSCALE,
                    accum_out=Zs4[r:r + 8, c2:c2 + 1])
            # transpose this batch's P block once both chunks are done
            nc.scalar.dma_start(out=PT[:, :, 16 * b:16 * b + 16],
                                in_=Pn[b * 32:b * 32 + 16, :], transpose=True)

        nc.vector.tensor_add(Zb[:, 0:1], Zs4[0:8, 0:1], Zs4[0:8, 1:2])
        nc.vector.tensor_add(Zb[:, 1:2], Zs4[32:40, 0:1], Zs4[32:40, 1:2])
        nc.vector.reciprocal(rzb, Zb)

        # cv2 batch 1: ACT work overlapping the He aggregation below
        conv_chunks(1, w2t, b2s, y2[1], accum=False)

        # ---------------- He (natural) -> edge -> whTT --------------------
        for b in range(BLOC):
            hep = psum.tile([8, 2, 128], FP32, tag="big", name="hep")
            for t in range(32):
                nc.tensor.matmul(
                    hep,
                    PT[:, t, 16 * b:16 * b + 8],
                    tl2[b][:, t::32, :],
                    start=(t == 0), stop=(t == 31))
            # He normalized by 1/Z on the way out of PSUM
            nc.vector.tensor_scalar_mul(Hs[b], hep, rzb[:, b:b + 1])
            # He^T via PE transpose (out = in.T against identity)
            for m in range(2):
                tp = psum.tile([128, 8], BF, tag="big", name="tp")
                nc.tensor.transpose(tp, Hs[b][:, m, :], ident)
                nc.vector.tensor_copy(heT[b][:, m, :], tp)
            hop = psum.tile([128, 2, 8], FP32, tag="big", name="hop")
            for mq in range(2):
                for kt in range(2):
                    nc.tensor.matmul(
                        hop[:, mq, :],
                        edgewt[:, kt, mq * 128:(mq + 1) * 128],
                        heT[b][:, kt, :],
                        start=(kt == 0), stop=(kt == 1))
            for mq in range(2):
                nc.scalar.activation(heoT[b][:, mq, :], hop[:, mq, :], AF.Gelu,
                                     bias=ebs[:, mq:mq + 1])
            # whTT rows [32b, 32b+8) = (Heo @ node_w^T) * (1/Z)
            wp = psum.tile([8, CH], FP32, tag="big", name="wp")
            for kt in range(2):
                nc.tensor.matmul(
                    wp,
                    heoT[b][:, kt, :],
                    nodewt[:, kt, :],
                    start=(kt == 0), stop=(kt == 1))
            nc.vector.tensor_scalar_mul(whTT[b * 32:b * 32 + 8, :], wp,
                                        rzb[:, b:b + 1])

        # ---------------- node-apply + gelu; m_out = tokens + gelu --------
        m_out = [xs_pool.tile([128, 2, N], BF, tag="xs", name="xs") for _ in range(BLOC)]
        for b in range(BLOC):
            for m in range(2):
                for c2 in range(NCHUNKS):
                    ps = psum.tile([128, NCH], FP32, tag="big", name="big")
                    for ns in range(NCH // NSUB):
                        nc.tensor.matmul(
                            ps[:, ns * NSUB:(ns + 1) * NSUB],
                            whTT[b * 32:b * 32 + 8, m * 128:(m + 1) * 128],
                            Pn[b * 32:b * 32 + 8, c2 * NCH + ns * NSUB: c2 * NCH + (ns + 1) * NSUB],
                            start=True, stop=True,
                            tile_position=(b * 32, 0))
                    gel = stage.tile([128, NCH], BF, tag="stage", name="stage")
                    nc.scalar.activation(gel, ps, AF.Gelu, bias=nbs[:, m:m + 1])
                    nc.vector.tensor_add(m_out[b][:, m, c2 * NCH:(c2 + 1) * NCH],
                                         gel, tokens[b][:, m, c2 * NCH:(c2 + 1) * NCH])

        # ---------------- cv3 + SiLU + store ------------------------------
        for b in range(BLOC):
            for m in range(2):
                for c2 in range(NCHUNKS):
                    ps = psum.tile([128, NCH], FP32, tag="big", name="big")
                    for kt in range(4):
                        rhs_t = m_out[b] if kt < 2 else y2[b]
                        for ns in range(NCH // NSUB):
                            nc.tensor.matmul(
                                ps[:, ns * NSUB:(ns + 1) * NSUB],
                                w3t[:, kt, m * 128:(m + 1) * 128],
                                rhs_t[:, kt % 2, c2 * NCH + ns * NSUB: c2 * NCH + (ns + 1) * NSUB],
                                start=(kt == 0), stop=(kt == 3))
                    ostg = stage.tile([128, NCH], BF, tag="stage", name="stage")
                    nc.scalar.activation(ostg, ps, AF.Silu, bias=b3s[:, m:m + 1])
                    nc.gpsimd.dma_start(
                        out=d["out"][b, m * 128:(m + 1) * 128, c2 * NCH:(c2 + 1) * NCH],
                        in_=ostg)


@functools.cache
def get_nc():
    nc = bacc.Bacc("TRN2", target_bir_lowering=False, debug=False,
                   enable_asserts=False, num_devices=NCORES)
    emit_kernel(nc)
    nc.finalize()
    return nc


def prep_inputs(inputs):
    """Host-side weight folding + dtype casts. Returns per-core input maps."""
    f32 = np.float32

    def fold(w, g, b, m, v):
        s = (g / np.sqrt(v + EPS)).astype(f32)
        return (np.asarray(w, f32) * s[:, None]), (b - m * s).astype(f32)

    W1, b1 = fold(inputs["cv1_w"], inputs["cv1_g"], inputs["cv1_b"], inputs["cv1_m"], inputs["cv1_v"])
    W2, b2 = fold(inputs["cv2_w"], inputs["cv2_g"], inputs["cv2_b"], inputs["cv2_m"], inputs["cv2_v"])
    W3, b3 = fold(inputs["cv3_w"], inputs["cv3_g"], inputs["cv3_b"], inputs["cv3_m"], inputs["cv3_v"])
    proto_eff = np.asarray(inputs["proto"], f32) + np.asarray(inputs["ctx_b"], f32).reshape(E, CH)
    pre_w = np.asarray(inputs["pre_w"], f32)
    ctx_w = np.asarray(inputs["ctx_w"], f32)

    # q^T = q0T + G @ ctx  (pre_w folded into ctx_w and proto on host)
    # q0[e, c] = sum_c2 proto_eff[e, c2] * pre_w[c2, c]
    q0 = proto_eff @ pre_w
    q0T = q0.reshape(E, 2, 128).transpose(2, 0, 1).reshape(128, 16)
    # G[(e, c), k] = sum_c2 pre_w[c2, c] * ctx_w[e*CH + c2, k]
    cw3 = ctx_w.reshape(E, CH, 2 * CH)
    G = np.einsum("xc,exk->eck", pre_w, cw3).reshape(E * CH, 2 * CH)

    shared = {
        "w1t": np.ascontiguousarray(W1.T).astype(BF16),
        "w2t": np.ascontiguousarray(W2.T).astype(BF16),
        "w3t": np.ascontiguousarray(W3.T).astype(BF16),
        "gwt": np.ascontiguousarray(G.T).astype(BF16),
        "q0t": np.ascontiguousarray(q0T).astype(BF16),
        "ident": np.eye(8, dtype=f32).astype(BF16),
        "edgewt": np.ascontiguousarray(np.asarray(inputs["edge_w"], f32).T).astype(BF16),
        "nodewt": np.ascontiguousarray(np.asarray(inputs["node_w"], f32).T).astype(BF16),
        "b1": b1, "b2": b2, "b3": b3,
        "eb": np.asarray(inputs["edge_b"], f32),
        "nb": np.asarray(inputs["node_b"], f32),
    }
    x = np.asarray(inputs["x"], f32).reshape(B, C1, N).astype(BF16)
    in_maps = []
    for c in range(NCORES):
        m = dict(shared)
        m["x"] = np.ascontiguousarray(x[c * BLOC:(c + 1) * BLOC])
        in_maps.append(m)
    return in_maps


def run(inputs, trace=False, **kw):
    nc = get_nc()
    in_maps = prep_inputs(inputs)
    res = run_bass_kernel_spmd(nc, in_maps, list(range(NCORES)), trace=trace, **kw)
    outs = [np.asarray(res.results[i]["out"], np.float32) for i in range(NCORES)]
    full = np.concatenate(outs, axis=0).reshape(B, C2, H, W)
    return full, res


def kernel(**inputs):
    out, _ = run(inputs, trace=False)
    return out
